# revision 1
# baseline (speedup 1.0000x reference)
"""CenterLoss on 8 Trainium2 NeuronCores.

Math: the reference builds the full (B, C) squared-distance matrix,
masks it to the one entry (i, labels[i]) per row, clamps AFTER masking
(so the C-1 masked zeros per row each become 1e-12), sums and divides
by B.  Only the gathered center rows matter:

    loss = (sum_i clip(||x_i - c_{l_i}||^2, 1e-12, 1e12)
            + B*(C-1)*1e-12) / B

Sharding: data-parallel over the batch — core k gets rows
[k*256, (k+1)*256) of x/labels and a full replica of centers in DRAM.
Each core gathers its 256 needed center rows with an indirect DMA
(reads 128 KB instead of 51 MB), computes per-row squared distances on
the vector engine, clamps, and writes the 256 distances out.  The host
sums the 8x256 partials and applies the constant clamp correction.
"""

import os

import numpy as np

BATCH = 2048
NUM_CLASSES = 100000
FEAT_DIM = 128
N_CORES = 8
ROWS_PER_CORE = BATCH // N_CORES  # 256
P = 128
TILES_PER_CORE = ROWS_PER_CORE // P  # 2

_CACHE = {}


def _build_raw():
    """Hand-synchronized raw-Bass kernel (no TileContext).

    Tile's entry barrier + exit drain/double-barrier/sem-clear cost
    ~10-13us of fixed overhead on a ~7us body. With manual semaphores the
    kernel is: labels DMA -> 2 indirect gathers (gpsimd), x DMA in
    parallel, a DVE chain (sub/sq/row-reduce/clamp) where tile 0's
    compute overlaps tile 1's gather, and one output DMA. Semaphores are
    cleared at the end so re-executing the same loaded NEFF stays correct.
    """
    from contextlib import ExitStack

    import concourse.bass as bass
    import concourse.mybir as mybir

    f32 = mybir.dt.float32
    i32 = mybir.dt.int32
    NT = TILES_PER_CORE
    D = FEAT_DIM

    # Row i of this core's shard maps to (partition, tile) = (i // NT,
    # i % NT): with row-index = p*NT + n every DMA's innermost dim is
    # contiguous in DRAM (tile-major row = n*P + p would stride it).
    nc = bass.Bass()
    x_d = nc.dram_tensor("x", [ROWS_PER_CORE, D], f32, kind="ExternalInput")
    lab_d = nc.dram_tensor("labels", [ROWS_PER_CORE, 1], i32, kind="ExternalInput")
    cen_d = nc.dram_tensor("centers", [NUM_CLASSES, D], f32, kind="ExternalInput")
    out_d = nc.dram_tensor("dists", [ROWS_PER_CORE, 1], f32, kind="ExternalOutput")

    with ExitStack() as ctx:
        x_all = ctx.enter_context(nc.sbuf_tensor([P, NT * D], f32))
        idx = ctx.enter_context(nc.sbuf_tensor([P, NT], i32))
        c_all = ctx.enter_context(nc.sbuf_tensor([P, NT * D], f32))
        dif = ctx.enter_context(nc.sbuf_tensor([P, NT * D], f32))
        sq = ctx.enter_context(nc.sbuf_tensor([P, NT * D], f32))
        s_all = ctx.enter_context(nc.sbuf_tensor([P, NT], f32))
        s_lab = ctx.enter_context(nc.semaphore("s_lab"))
        s_x = ctx.enter_context(nc.semaphore("s_x"))
        s_g = ctx.enter_context(nc.semaphore("s_g"))
        s_v = ctx.enter_context(nc.semaphore("s_v"))
        s_out = ctx.enter_context(nc.semaphore("s_out"))
        s_d = ctx.enter_context(nc.semaphore("s_d"))

        # Semaphores are NOT guaranteed zero at NEFF load (a prior kernel
        # or interrupted execution can leave residue, which makes waits
        # pass early and silently corrupts rows). Clear them, then sync
        # all engines with the NRT pseudo barrier (runtime-expanded, so it
        # does not itself depend on bass sems) — the same pattern Bass's
        # lowering preamble uses.
        for s in (s_x, s_g, s_v, s_out, s_d):
            nc.gpsimd.sem_clear(s)
        # The labels load is the long pole (DMA + ~1.5us completion-sem
        # latency gate the gathers), so issue it BEFORE the barrier: sync
        # clears s_lab itself (same-engine order makes clear-then-inc
        # race-free) and the consumer's wait sits behind the barrier.
        nc.sync.sem_clear(s_lab)
        nc.sync.dma_start(
            out=idx[:], in_=lab_d[:].rearrange("(p n) o -> p (n o)", n=NT)
        ).then_inc(s_lab, 16)
        nc._nrt_pseudo_barrier()

        # Flat per-engine streams, no nc.Block: the Block's per-engine
        # bodies add COMPARE_BRANCHes and an exit all-engine barrier
        # (~2us). Emission order below IS each engine's program order.

        # sync: x in, then (after DVE finishes) results out
        nc.sync.dma_start(
            out=x_all[:].rearrange("p (n d) -> p n d", n=NT),
            in_=x_d[:].rearrange("(p n) d -> p n d", n=NT),
        ).then_inc(s_x, 16)

        # gpsimd: two gathers of 128 rows, not one of 256 — consecutive
        # indirect DMAs round-robin onto different SWDGE queues, so their
        # per-descriptor payload processing (~23ns/row/queue) overlaps. A
        # single 256-row gather serializes all payload on one queue
        # (+6.5us measured). Offset tables must be SBUF.
        nc.gpsimd.wait_ge(s_lab, 16)
        for t in range(NT):
            nc.gpsimd.indirect_dma_start(
                out=c_all[:, t * D : (t + 1) * D],
                out_offset=None,
                in_=cen_d[:],
                in_offset=bass.IndirectOffsetOnAxis(ap=idx[:, t : t + 1], axis=0),
            ).then_inc(s_g, 16)

        # vector: DVE RAW hazards between back-to-back ops are real (the
        # pipe flush only covers output hazards), so dependent ops chain
        # through the s_d self-semaphore. Batched whole-width ops (one
        # sub/mul/reduce over both tiles) halve the per-element DVE cost
        # vs per-tile ops. The torch clamp clip(d, 1e-12, 1e12) is applied
        # on the host: d here is a direct sum of squares (>= 0, and
        # ~144..384 for this data), so a device-side clamp cannot bind.
        nc.vector.wait_ge(s_x, 16)
        nc.vector.wait_ge(s_g, 16 * NT)
        nc.vector.tensor_tensor(
            out=dif[:],
            in0=x_all[:],
            in1=c_all[:],
            op=mybir.AluOpType.subtract,
        ).then_inc(s_d, 1)
        nc.vector.wait_ge(s_d, 1)
        nc.vector.tensor_tensor(
            out=sq[:], in0=dif[:], in1=dif[:], op=mybir.AluOpType.mult
        ).then_inc(s_d, 1)
        nc.vector.wait_ge(s_d, 2)
        nc.vector.tensor_reduce(
            out=s_all[:],
            in_=sq[:].rearrange("p (n d) -> p n d", n=NT),
            axis=mybir.AxisListType.X,
            op=mybir.AluOpType.add,
        ).then_inc(s_v, 1)

        # sync tail: results out once DVE signals, then one cheap drain so
        # the engines halt only after the output DMA lands. (No exit sem
        # clears needed — the entry clears make each execution
        # self-correcting; gpsimd's expensive dge_drain is skipped, its
        # queues are proven drained via s_g.)
        nc.sync.wait_ge(s_v, 1)
        nc.sync.dma_start(
            out=out_d[:].rearrange("(p n) o -> p (n o)", n=NT), in_=s_all[:]
        ).then_inc(s_out, 16)
        nc.sync.drain()

    return nc


def _build_v2():
    """v2: same dataflow as _build_raw, with three scheduling changes.

    1. The x load moves to the scalar engine's HWDGE queue so the sync
       queue carries only the 1 KB labels DMA - its payload and
       completion semaphore (the gather's gate) fire sooner.
    2. Per-tile pipeline across two engines: DVE subtracts, the scalar
       engine squares + row-reduces in one activation (accum_out), so
       tile 0's compute overlaps tile 1's gather payload. A pre-barrier
       dummy activation hides the ~1.3us Square table load.
    3. The output is written as plain [128, 2] with no rearrange (the
       host sums all entries, so element order is irrelevant) by the
       scalar engine itself - the producer issues the store, no
       cross-engine handoff on the tail.
    """
    from contextlib import ExitStack

    import concourse.bass as bass
    import concourse.mybir as mybir

    f32 = mybir.dt.float32
    i32 = mybir.dt.int32
    NT = TILES_PER_CORE
    D = FEAT_DIM

    nc = bass.Bass()
    x_d = nc.dram_tensor("x", [ROWS_PER_CORE, D], f32, kind="ExternalInput")
    lab_d = nc.dram_tensor("labels", [ROWS_PER_CORE, 1], i32, kind="ExternalInput")
    cen_d = nc.dram_tensor("centers", [NUM_CLASSES, D], f32, kind="ExternalInput")
    out_d = nc.dram_tensor("dists", [P, NT], f32, kind="ExternalOutput")

    with ExitStack() as ctx:
        x_all = ctx.enter_context(nc.sbuf_tensor([P, NT * D], f32))
        idx = ctx.enter_context(nc.sbuf_tensor([P, NT], i32))
        c_all = ctx.enter_context(nc.sbuf_tensor([P, NT * D], f32))
        dif = ctx.enter_context(nc.sbuf_tensor([P, NT * D], f32))
        sq = ctx.enter_context(nc.sbuf_tensor([P, NT * D], f32))
        s_all = ctx.enter_context(nc.sbuf_tensor([P, NT], f32))
        s_lab = ctx.enter_context(nc.semaphore("s_lab"))
        s_x = ctx.enter_context(nc.semaphore("s_x"))
        s_g = ctx.enter_context(nc.semaphore("s_g"))
        s_out = ctx.enter_context(nc.semaphore("s_out"))
        s_d = ctx.enter_context(nc.semaphore("s_d"))

        # Entry clears: each sem is cleared on the engine that issues its
        # first increment (same-engine order makes clear-then-inc safe);
        # every cross-engine wait sits behind the NRT barrier.
        nc.sync.sem_clear(s_lab)
        nc.sync.dma_start(
            out=idx[:], in_=lab_d[:].rearrange("(p n) o -> p (n o)", n=NT)
        ).then_inc(s_lab, 16)
        nc.scalar.sem_clear(s_x)
        nc.scalar.sem_clear(s_out)
        nc.scalar.dma_start(
            out=x_all[:].rearrange("p (n d) -> p n d", n=NT),
            in_=x_d[:].rearrange("(p n) d -> p n d", n=NT),
        ).then_inc(s_x, 16)
        # Dummy activation: loads the Square function table (~1.3us)
        # while the gathers are still waiting on labels.
        nc.scalar.activation(
            out=sq[:, 0:1],
            in_=s_all[:, 0:1],
            func=mybir.ActivationFunctionType.Square,
        )
        nc.gpsimd.sem_clear(s_g)
        nc.vector.sem_clear(s_d)
        nc._nrt_pseudo_barrier()

        # gpsimd: two 128-row gathers (round-robin across SWDGE queues)
        nc.gpsimd.wait_ge(s_lab, 16)
        for t in range(NT):
            nc.gpsimd.indirect_dma_start(
                out=c_all[:, t * D : (t + 1) * D],
                out_offset=None,
                in_=cen_d[:],
                in_offset=bass.IndirectOffsetOnAxis(ap=idx[:, t : t + 1], axis=0),
            ).then_inc(s_g, 16)

        # vector: per-tile subtract, pipelined with the second gather's
        # payload; the scalar engine squares + row-reduces each tile.
        nc.vector.wait_ge(s_x, 16)
        for t in range(NT):
            cols = slice(t * D, (t + 1) * D)
            nc.vector.wait_ge(s_g, 16 * (t + 1))
            nc.vector.tensor_tensor(
                out=dif[:, cols],
                in0=x_all[:, cols],
                in1=c_all[:, cols],
                op=mybir.AluOpType.subtract,
            ).then_inc(s_d, 1)

        # scalar: square + row-sum per tile, then store dists itself.
        for t in range(NT):
            cols = slice(t * D, (t + 1) * D)
            nc.scalar.wait_ge(s_d, t + 1)
            nc.scalar.activation(
                out=sq[:, cols],
                in_=dif[:, cols],
                func=mybir.ActivationFunctionType.Square,
                accum_out=s_all[:, t : t + 1],
            )
        nc.scalar.dma_start(out=out_d[:], in_=s_all[:]).then_inc(s_out, 16)
        nc.scalar.drain()
        nc.sync.drain()

    return nc


def _build_v3():
    """v3: critical path = labels -> gather -> subtract -> square-reduce.

    Changes vs v2 (from its trace):
    - labels load moves to gpsimd's software DGE: the whole
      clear -> dma -> wait chain is same-engine (no barrier dependency)
      and the SWDGE completion semaphore propagates in ~0.5us where the
      sync HWDGE one took ~1.9us.
    - the dummy Square-table-load activation moves AFTER the barrier: on
      the scalar DSP it kept the barrier from completing (~1us), and
      post-barrier it hides fully under the gathers.
    - x load issues post-barrier on the scalar HWDGE, so the labels
      payload has the DMA engines to itself.
    - act1 chains to the output DMA through a self-semaphore: v2's trace
      showed the out descriptor-gen starting before the accumulator
      writeback (a real race, seen as 4e-5 rel err on one run).
    """
    from contextlib import ExitStack

    import concourse.bass as bass
    import concourse.mybir as mybir

    f32 = mybir.dt.float32
    i32 = mybir.dt.int32
    NT = TILES_PER_CORE
    D = FEAT_DIM

    nc = bass.Bass()
    x_d = nc.dram_tensor("x", [ROWS_PER_CORE, D], f32, kind="ExternalInput")
    lab_d = nc.dram_tensor("labels", [ROWS_PER_CORE, 1], i32, kind="ExternalInput")
    cen_d = nc.dram_tensor("centers", [NUM_CLASSES, D], f32, kind="ExternalInput")
    out_d = nc.dram_tensor("dists", [P, NT], f32, kind="ExternalOutput")

    with ExitStack() as ctx:
        x_all = ctx.enter_context(nc.sbuf_tensor([P, NT * D], f32))
        idx = ctx.enter_context(nc.sbuf_tensor([P, NT], i32))
        c_all = ctx.enter_context(nc.sbuf_tensor([P, NT * D], f32))
        dif = ctx.enter_context(nc.sbuf_tensor([P, NT * D], f32))
        sq = ctx.enter_context(nc.sbuf_tensor([P, NT * D], f32))
        s_all = ctx.enter_context(nc.sbuf_tensor([P, NT], f32))
        s_lab = ctx.enter_context(nc.semaphore("s_lab"))
        s_x = ctx.enter_context(nc.semaphore("s_x"))
        s_g = ctx.enter_context(nc.semaphore("s_g"))
        s_out = ctx.enter_context(nc.semaphore("s_out"))
        s_d = ctx.enter_context(nc.semaphore("s_d"))
        s_a = ctx.enter_context(nc.semaphore("s_a"))

        # gpsimd owns the whole labels -> gather chain; clears first.
        nc.gpsimd.sem_clear(s_lab)
        nc.gpsimd.sem_clear(s_g)
        nc.gpsimd.dma_start(
            out=idx[:], in_=lab_d[:].rearrange("(p n) o -> p (n o)", n=NT)
        ).then_inc(s_lab, 16)
        nc.scalar.sem_clear(s_x)
        nc.scalar.sem_clear(s_out)
        nc.scalar.sem_clear(s_a)
        nc.vector.sem_clear(s_d)
        nc._nrt_pseudo_barrier()

        # gpsimd: two 128-row gathers as soon as the labels land.
        nc.gpsimd.wait_ge(s_lab, 16)
        for t in range(NT):
            nc.gpsimd.indirect_dma_start(
                out=c_all[:, t * D : (t + 1) * D],
                out_offset=None,
                in_=cen_d[:],
                in_offset=bass.IndirectOffsetOnAxis(ap=idx[:, t : t + 1], axis=0),
            ).then_inc(s_g, 16)

        # scalar: x load + Square-table preload, both hidden under the
        # gathers; x is needed by the DVE only at first subtract.
        nc.scalar.dma_start(
            out=x_all[:].rearrange("p (n d) -> p n d", n=NT),
            in_=x_d[:].rearrange("(p n) d -> p n d", n=NT),
        ).then_inc(s_x, 16)
        nc.scalar.activation(
            out=sq[:, 0:1],
            in_=s_all[:, 0:1],
            func=mybir.ActivationFunctionType.Square,
        )

        # vector: per-tile subtract, pipelined with the second gather.
        nc.vector.wait_ge(s_x, 16)
        for t in range(NT):
            cols = slice(t * D, (t + 1) * D)
            nc.vector.wait_ge(s_g, 16 * (t + 1))
            nc.vector.tensor_tensor(
                out=dif[:, cols],
                in0=x_all[:, cols],
                in1=c_all[:, cols],
                op=mybir.AluOpType.subtract,
            ).then_inc(s_d, 1)

        # scalar: square + row-sum per tile; the self-semaphore s_a
        # orders the store after act1's accumulator writeback.
        for t in range(NT):
            cols = slice(t * D, (t + 1) * D)
            nc.scalar.wait_ge(s_d, t + 1)
            nc.scalar.activation(
                out=sq[:, cols],
                in_=dif[:, cols],
                func=mybir.ActivationFunctionType.Square,
                accum_out=s_all[:, t : t + 1],
            ).then_inc(s_a, 1)
        nc.scalar.wait_ge(s_a, NT)
        nc.scalar.dma_start(out=out_d[:], in_=s_all[:]).then_inc(s_out, 16)
        nc.scalar.drain()
        nc.sync.drain()

    return nc


def _build_v4():
    """v4: one merged 256-row gather driven by a flat offset list.

    labels land as [1, 256] in partition 0 (single-descriptor DMA); the
    indirect gather reads all 256 offsets from that one partition and
    writes c_all[p, n*D:...] = centers[labels[p*2+n]] (offset iteration
    order matches the dest AP's p-major block order, which is exactly
    DRAM row order). One 256-wide subtract + one Square activation whose
    accumulator gives per-partition row-PAIR sums - fine, the host only
    needs the total (clamp can't bind: dists are sums of squares,
    O(100), far inside [1e-12, 1e12]).
    """
    from contextlib import ExitStack

    import concourse.bass as bass
    import concourse.mybir as mybir

    f32 = mybir.dt.float32
    i32 = mybir.dt.int32
    NT = TILES_PER_CORE
    D = FEAT_DIM

    nc = bass.Bass()
    x_d = nc.dram_tensor("x", [ROWS_PER_CORE, D], f32, kind="ExternalInput")
    lab_d = nc.dram_tensor("labels", [ROWS_PER_CORE, 1], i32, kind="ExternalInput")
    cen_d = nc.dram_tensor("centers", [NUM_CLASSES, D], f32, kind="ExternalInput")
    out_d = nc.dram_tensor("dists", [P, 1], f32, kind="ExternalOutput")

    with ExitStack() as ctx:
        x_all = ctx.enter_context(nc.sbuf_tensor([P, NT * D], f32))
        idx1 = ctx.enter_context(nc.sbuf_tensor([1, ROWS_PER_CORE], i32))
        c_all = ctx.enter_context(nc.sbuf_tensor([P, NT * D], f32))
        dif = ctx.enter_context(nc.sbuf_tensor([P, NT * D], f32))
        sq = ctx.enter_context(nc.sbuf_tensor([P, NT * D], f32))
        s_all = ctx.enter_context(nc.sbuf_tensor([P, 1], f32))
        s_lab = ctx.enter_context(nc.semaphore("s_lab"))
        s_x = ctx.enter_context(nc.semaphore("s_x"))
        s_g = ctx.enter_context(nc.semaphore("s_g"))
        s_out = ctx.enter_context(nc.semaphore("s_out"))
        s_d = ctx.enter_context(nc.semaphore("s_d"))
        s_a = ctx.enter_context(nc.semaphore("s_a"))

        # labels via sync HWDGE pre-barrier as a single descriptor into
        # partition 0. (gpsimd SWDGE is NOT an option: the NRT barrier
        # fences the issuing engine's in-flight SWDGE DMA, ~+3us.)
        nc.sync.sem_clear(s_lab)
        nc.sync.dma_start(
            out=idx1[:], in_=lab_d[:].rearrange("(o r) o2 -> o (r o2)", o=1)
        ).then_inc(s_lab, 16)
        nc.gpsimd.sem_clear(s_g)
        nc.scalar.sem_clear(s_x)
        nc.scalar.sem_clear(s_out)
        nc.scalar.sem_clear(s_a)
        nc.vector.sem_clear(s_d)
        nc._nrt_pseudo_barrier()

        # one 256-row gather: dest blocks iterate (p, n) p-major, offset
        # list iterates j = p*NT + n - the same linear order.
        nc.gpsimd.wait_ge(s_lab, 16)
        nc.gpsimd.indirect_dma_start(
            out=c_all[:].rearrange("p (n d) -> p n d", n=NT),
            out_offset=None,
            in_=cen_d[:],
            in_offset=bass.IndirectOffsetOnAxis(ap=idx1[:], axis=0),
        ).then_inc(s_g, 16)

        # scalar: x load + Square-table preload, hidden under the gather.
        nc.scalar.dma_start(
            out=x_all[:].rearrange("p (n d) -> p n d", n=NT),
            in_=x_d[:].rearrange("(p n) d -> p n d", n=NT),
        ).then_inc(s_x, 16)
        nc.scalar.activation(
            out=sq[:, 0:1],
            in_=s_all[:, 0:1],
            func=mybir.ActivationFunctionType.Square,
        )

        # vector: one full-width subtract.
        nc.vector.wait_ge(s_x, 16)
        nc.vector.wait_ge(s_g, 16)
        nc.vector.tensor_tensor(
            out=dif[:],
            in0=x_all[:],
            in1=c_all[:],
            op=mybir.AluOpType.subtract,
        ).then_inc(s_d, 1)

        # scalar: square + row-pair sum, then store [128, 1].
        nc.scalar.wait_ge(s_d, 1)
        nc.scalar.activation(
            out=sq[:],
            in_=dif[:],
            func=mybir.ActivationFunctionType.Square,
            accum_out=s_all[:],
        ).then_inc(s_a, 1)
        nc.scalar.wait_ge(s_a, 1)
        nc.scalar.dma_start(out=out_d[:], in_=s_all[:]).then_inc(s_out, 16)
        nc.scalar.drain()
        nc.sync.drain()

    return nc



def _build_v5(dram_offsets=False, out_on_sync=False):
    """v5: v2 dataflow with the scheduling fixes learned from traces.

    - x DMA + Square-table preload moved POST-barrier on scalar: the
      pre-barrier dummy activation kept the barrier from completing
      (~1us) and the x payload contended with the labels payload.
    - act1 chains to the output DMA through a self-semaphore (v2 raced).
    - labels stay on sync HWDGE pre-barrier ([128, NT] layout, per-
      partition offset tables - the only gather shape proven on HW).
    - dram_offsets=True: skip the labels DMA entirely and point the
      gather offset tables at DRAM (experimental).
    """
    from contextlib import ExitStack

    import concourse.bass as bass
    import concourse.mybir as mybir

    f32 = mybir.dt.float32
    i32 = mybir.dt.int32
    NT = TILES_PER_CORE
    D = FEAT_DIM

    nc = bass.Bass()
    x_d = nc.dram_tensor("x", [ROWS_PER_CORE, D], f32, kind="ExternalInput")
    lab_d = nc.dram_tensor("labels", [ROWS_PER_CORE, 1], i32, kind="ExternalInput")
    cen_d = nc.dram_tensor("centers", [NUM_CLASSES, D], f32, kind="ExternalInput")
    out_d = nc.dram_tensor("dists", [P, NT], f32, kind="ExternalOutput")

    with ExitStack() as ctx:
        x_all = ctx.enter_context(nc.sbuf_tensor([P, NT * D], f32))
        idx = ctx.enter_context(nc.sbuf_tensor([P, NT], i32))
        c_all = ctx.enter_context(nc.sbuf_tensor([P, NT * D], f32))
        dif = ctx.enter_context(nc.sbuf_tensor([P, NT * D], f32))
        sq = ctx.enter_context(nc.sbuf_tensor([P, NT * D], f32))
        s_all = ctx.enter_context(nc.sbuf_tensor([P, NT], f32))
        s_lab = ctx.enter_context(nc.semaphore("s_lab"))
        s_x = ctx.enter_context(nc.semaphore("s_x"))
        s_g = ctx.enter_context(nc.semaphore("s_g"))
        s_out = ctx.enter_context(nc.semaphore("s_out"))
        s_d = ctx.enter_context(nc.semaphore("s_d"))
        s_a = ctx.enter_context(nc.semaphore("s_a"))

        if not dram_offsets:
            nc.sync.sem_clear(s_lab)
            nc.sync.dma_start(
                out=idx[:], in_=lab_d[:].rearrange("(p n) o -> p (n o)", n=NT)
            ).then_inc(s_lab, 16)
        nc.scalar.sem_clear(s_x)
        nc.scalar.sem_clear(s_out)
        nc.scalar.sem_clear(s_a)
        if dram_offsets:
            # no labels payload to contend with - issue x pre-barrier so
            # its completion sem comfortably beats the first subtract.
            nc.scalar.dma_start(
                out=x_all[:].rearrange("p (n d) -> p n d", n=NT),
                in_=x_d[:].rearrange("(p n) d -> p n d", n=NT),
            ).then_inc(s_x, 16)
        nc.gpsimd.sem_clear(s_g)
        nc.vector.sem_clear(s_d)
        nc._nrt_pseudo_barrier()

        # gpsimd: two 128-row gathers as soon as the labels land.
        if not dram_offsets:
            nc.gpsimd.wait_ge(s_lab, 16)
        lab_view = lab_d[:].rearrange("(p n) o -> p (n o)", n=NT)
        for t in range(NT):
            off_ap = (
                lab_view[:, t : t + 1] if dram_offsets else idx[:, t : t + 1]
            )
            nc.gpsimd.indirect_dma_start(
                out=c_all[:, t * D : (t + 1) * D],
                out_offset=None,
                in_=cen_d[:],
                in_offset=bass.IndirectOffsetOnAxis(ap=off_ap, axis=0),
            ).then_inc(s_g, 16)

        # scalar: x load + Square-table preload, hidden under the gathers.
        if not dram_offsets:
            nc.scalar.dma_start(
                out=x_all[:].rearrange("p (n d) -> p n d", n=NT),
                in_=x_d[:].rearrange("(p n) d -> p n d", n=NT),
            ).then_inc(s_x, 16)
        nc.scalar.activation(
            out=sq[:, 0:1],
            in_=s_all[:, 0:1],
            func=mybir.ActivationFunctionType.Square,
        )

        # vector: per-tile subtract, pipelined with the second gather.
        nc.vector.wait_ge(s_x, 16)
        for t in range(NT):
            cols = slice(t * D, (t + 1) * D)
            nc.vector.wait_ge(s_g, 16 * (t + 1))
            nc.vector.tensor_tensor(
                out=dif[:, cols],
                in0=x_all[:, cols],
                in1=c_all[:, cols],
                op=mybir.AluOpType.subtract,
            ).then_inc(s_d, 1)

        # scalar: square + row-sum per tile, store after s_a (act1 wb).
        for t in range(NT):
            cols = slice(t * D, (t + 1) * D)
            nc.scalar.wait_ge(s_d, t + 1)
            nc.scalar.activation(
                out=sq[:, cols],
                in_=dif[:, cols],
                func=mybir.ActivationFunctionType.Square,
                accum_out=s_all[:, t : t + 1],
            ).then_inc(s_a, 1)
        if out_on_sync:
            nc.sync.wait_ge(s_a, NT)
            nc.sync.dma_start(out=out_d[:], in_=s_all[:]).then_inc(s_out, 16)
        else:
            nc.scalar.wait_ge(s_a, NT)
            nc.scalar.dma_start(out=out_d[:], in_=s_all[:]).then_inc(s_out, 16)
        nc.scalar.drain()
        nc.sync.drain()

    return nc


def _build_v9():
    return _build_v5(out_on_sync=True)


def _build_v6():
    return _build_v5(dram_offsets=True)



def _build_v7():
    """v7: v5 but ONE merged 256-row gather ([128,2] per-partition offset
    pairs). Saves one ~1.1us INDIRECT1D issue + the 310ns gap if the
    payload of a single indirect DMA spreads across DMA engines."""
    from contextlib import ExitStack

    import concourse.bass as bass
    import concourse.mybir as mybir

    f32 = mybir.dt.float32
    i32 = mybir.dt.int32
    NT = TILES_PER_CORE
    D = FEAT_DIM

    nc = bass.Bass()
    x_d = nc.dram_tensor("x", [ROWS_PER_CORE, D], f32, kind="ExternalInput")
    lab_d = nc.dram_tensor("labels", [ROWS_PER_CORE, 1], i32, kind="ExternalInput")
    cen_d = nc.dram_tensor("centers", [NUM_CLASSES, D], f32, kind="ExternalInput")
    out_d = nc.dram_tensor("dists", [P, 1], f32, kind="ExternalOutput")

    with ExitStack() as ctx:
        x_all = ctx.enter_context(nc.sbuf_tensor([P, NT * D], f32))
        idx = ctx.enter_context(nc.sbuf_tensor([P, NT], i32))
        c_all = ctx.enter_context(nc.sbuf_tensor([P, NT * D], f32))
        dif = ctx.enter_context(nc.sbuf_tensor([P, NT * D], f32))
        sq = ctx.enter_context(nc.sbuf_tensor([P, NT * D], f32))
        s_all = ctx.enter_context(nc.sbuf_tensor([P, NT], f32))
        s_lab = ctx.enter_context(nc.semaphore("s_lab"))
        s_x = ctx.enter_context(nc.semaphore("s_x"))
        s_g = ctx.enter_context(nc.semaphore("s_g"))
        s_out = ctx.enter_context(nc.semaphore("s_out"))
        s_d = ctx.enter_context(nc.semaphore("s_d"))
        s_a = ctx.enter_context(nc.semaphore("s_a"))

        nc.sync.sem_clear(s_lab)
        nc.sync.dma_start(
            out=idx[:], in_=lab_d[:].rearrange("(p n) o -> p (n o)", n=NT)
        ).then_inc(s_lab, 16)
        nc.scalar.sem_clear(s_x)
        nc.scalar.sem_clear(s_out)
        nc.scalar.sem_clear(s_a)
        nc.gpsimd.sem_clear(s_g)
        nc.vector.sem_clear(s_d)
        nc._nrt_pseudo_barrier()

        # one 256-row gather: offset (p, n) pairs with dest block (p, n)
        nc.gpsimd.wait_ge(s_lab, 16)
        nc.gpsimd.indirect_dma_start(
            out=c_all[:],
            out_offset=None,
            in_=cen_d[:],
            in_offset=bass.IndirectOffsetOnAxis(ap=idx[:], axis=0),
        ).then_inc(s_g, 16)

        nc.scalar.dma_start(
            out=x_all[:].rearrange("p (n d) -> p n d", n=NT),
            in_=x_d[:].rearrange("(p n) d -> p n d", n=NT),
        ).then_inc(s_x, 16)
        nc.scalar.activation(
            out=sq[:, 0:1],
            in_=s_all[:, 0:1],
            func=mybir.ActivationFunctionType.Square,
        )

        nc.vector.wait_ge(s_x, 16)
        nc.vector.wait_ge(s_g, 16)
        nc.vector.tensor_tensor(
            out=dif[:],
            in0=x_all[:],
            in1=c_all[:],
            op=mybir.AluOpType.subtract,
        ).then_inc(s_d, 1)

        nc.scalar.wait_ge(s_d, 1)
        nc.scalar.activation(
            out=sq[:],
            in_=dif[:],
            func=mybir.ActivationFunctionType.Square,
            accum_out=s_all[:, 0:1],
        ).then_inc(s_a, 1)
        nc.scalar.wait_ge(s_a, 1)
        nc.scalar.dma_start(out=out_d[:], in_=s_all[:, 0:1]).then_inc(s_out, 16)
        nc.scalar.drain()
        nc.sync.drain()

    return nc



def _build_v8():
    """v8: v5 + tail restructure. The two per-tile results are stored by
    the idle sync engine as separate [128,1] outputs: dists0 is issued
    right after act0 (fully hidden under tile 1's compute) and dists1
    right after act1. SP HWDGE has the shortest issue (565ns) + DGE
    delay (650ns), and scalar halts immediately after act1."""
    from contextlib import ExitStack

    import concourse.bass as bass
    import concourse.mybir as mybir

    f32 = mybir.dt.float32
    i32 = mybir.dt.int32
    NT = TILES_PER_CORE
    D = FEAT_DIM

    nc = bass.Bass()
    x_d = nc.dram_tensor("x", [ROWS_PER_CORE, D], f32, kind="ExternalInput")
    lab_d = nc.dram_tensor("labels", [ROWS_PER_CORE, 1], i32, kind="ExternalInput")
    cen_d = nc.dram_tensor("centers", [NUM_CLASSES, D], f32, kind="ExternalInput")
    out_ds = [
        nc.dram_tensor(f"dists{t}", [P, 1], f32, kind="ExternalOutput")
        for t in range(NT)
    ]

    with ExitStack() as ctx:
        x_all = ctx.enter_context(nc.sbuf_tensor([P, NT * D], f32))
        idx = ctx.enter_context(nc.sbuf_tensor([P, NT], i32))
        c_all = ctx.enter_context(nc.sbuf_tensor([P, NT * D], f32))
        dif = ctx.enter_context(nc.sbuf_tensor([P, NT * D], f32))
        sq = ctx.enter_context(nc.sbuf_tensor([P, NT * D], f32))
        s_all = ctx.enter_context(nc.sbuf_tensor([P, NT], f32))
        s_lab = ctx.enter_context(nc.semaphore("s_lab"))
        s_x = ctx.enter_context(nc.semaphore("s_x"))
        s_g = ctx.enter_context(nc.semaphore("s_g"))
        s_out = ctx.enter_context(nc.semaphore("s_out"))
        s_d = ctx.enter_context(nc.semaphore("s_d"))
        s_a = ctx.enter_context(nc.semaphore("s_a"))

        nc.sync.sem_clear(s_lab)
        nc.sync.dma_start(
            out=idx[:], in_=lab_d[:].rearrange("(p n) o -> p (n o)", n=NT)
        ).then_inc(s_lab, 16)
        nc.sync.sem_clear(s_out)
        nc.scalar.sem_clear(s_x)
        nc.scalar.sem_clear(s_a)
        nc.gpsimd.sem_clear(s_g)
        nc.vector.sem_clear(s_d)
        nc._nrt_pseudo_barrier()

        # gpsimd: two 128-row gathers as soon as the labels land.
        nc.gpsimd.wait_ge(s_lab, 16)
        for t in range(NT):
            nc.gpsimd.indirect_dma_start(
                out=c_all[:, t * D : (t + 1) * D],
                out_offset=None,
                in_=cen_d[:],
                in_offset=bass.IndirectOffsetOnAxis(ap=idx[:, t : t + 1], axis=0),
            ).then_inc(s_g, 16)

        # scalar: x load + Square-table preload, hidden under the gathers.
        nc.scalar.dma_start(
            out=x_all[:].rearrange("p (n d) -> p n d", n=NT),
            in_=x_d[:].rearrange("(p n) d -> p n d", n=NT),
        ).then_inc(s_x, 16)
        nc.scalar.activation(
            out=sq[:, 0:1],
            in_=s_all[:, 0:1],
            func=mybir.ActivationFunctionType.Square,
        )

        # vector: per-tile subtract, pipelined with the second gather.
        nc.vector.wait_ge(s_x, 16)
        for t in range(NT):
            cols = slice(t * D, (t + 1) * D)
            nc.vector.wait_ge(s_g, 16 * (t + 1))
            nc.vector.tensor_tensor(
                out=dif[:, cols],
                in0=x_all[:, cols],
                in1=c_all[:, cols],
                op=mybir.AluOpType.subtract,
            ).then_inc(s_d, 1)

        # scalar: square + row-sum per tile.
        for t in range(NT):
            cols = slice(t * D, (t + 1) * D)
            nc.scalar.wait_ge(s_d, t + 1)
            nc.scalar.activation(
                out=sq[:, cols],
                in_=dif[:, cols],
                func=mybir.ActivationFunctionType.Square,
                accum_out=s_all[:, t : t + 1],
            ).then_inc(s_a, 1)

        # sync: store each tile's dists as soon as its act retires.
        for t in range(NT):
            nc.sync.wait_ge(s_a, t + 1)
            nc.sync.dma_start(
                out=out_ds[t][:], in_=s_all[:, t : t + 1]
            ).then_inc(s_out, 16)
        nc.sync.drain()
        nc.scalar.drain()

    return nc


def _build_bass():
    import concourse.bass as bass
    import concourse.bacc as bacc
    import concourse.mybir as mybir
    from concourse.tile import TileContext

    f32 = mybir.dt.float32
    i32 = mybir.dt.int32

    # Bacc (not raw Bass): its compile passes redistribute semaphore waits
    # that exceed an instruction's sync-wait slots (e.g. the kernel-tail
    # drain), which raw Bass leaves to fail in walrus codegen.
    nc = bacc.Bacc("TRN2", target_bir_lowering=False, debug=False)
    x_d = nc.dram_tensor("x", [ROWS_PER_CORE, FEAT_DIM], f32, kind="ExternalInput")
    lab_d = nc.dram_tensor("labels", [ROWS_PER_CORE, 1], i32, kind="ExternalInput")
    cen_d = nc.dram_tensor(
        "centers", [NUM_CLASSES, FEAT_DIM], f32, kind="ExternalInput"
    )
    out_d = nc.dram_tensor(
        "dists", [TILES_PER_CORE, P], f32, kind="ExternalOutput"
    )

    NT = TILES_PER_CORE
    # Hardware wait-slot limits shape this kernel:
    #  - a TensorTensor encodes ONE sync wait, so both of its operands must
    #    be produced on the DVE (same-sem waits merge into one threshold);
    #  - the kernel-tail Drain encodes ~8 waits, so every extra DMA queue
    #    (one semaphore each) counts — batch all loads/stores into one DMA.
    with TileContext(nc) as tc:
        with tc.tile_pool(name="pool", bufs=2) as pool, tc.tile_pool(
            name="persist", bufs=1
        ) as persist:
            # One DMA per input: x as [128, NT*128], labels as [128, NT]
            x_all = persist.tile([P, NT * FEAT_DIM], f32, tag="x_all")
            nc.sync.dma_start(
                out=x_all[:].rearrange("p (n d) -> p n d", n=NT),
                in_=x_d[:].rearrange("(n p) d -> p n d", p=P),
            )
            idx_all = persist.tile([P, NT], i32, tag="idx_all")
            nc.sync.dma_start(
                out=idx_all[:],
                in_=lab_d[:].rearrange("(n p) o -> p (n o)", p=P),
            )
            # Whole-x DVE copy: downstream TensorTensors read it via the DVE
            # self-semaphore instead of a second DMA semaphore.
            xb = persist.tile([P, NT * FEAT_DIM], f32, tag="xb")
            nc.vector.tensor_copy(out=xb[:], in_=x_all[:])
            s_all = persist.tile([P, NT], f32, tag="s_all")

            for t in range(NT):
                cols = slice(t * FEAT_DIM, (t + 1) * FEAT_DIM)
                c_t = pool.tile([P, FEAT_DIM], f32, tag="c")
                nc.gpsimd.indirect_dma_start(
                    out=c_t[:],
                    out_offset=None,
                    in_=cen_d[:],
                    in_offset=bass.IndirectOffsetOnAxis(
                        ap=idx_all[:, t : t + 1], axis=0
                    ),
                )
                diff = pool.tile([P, FEAT_DIM], f32, tag="diff")
                nc.vector.tensor_copy(out=diff[:], in_=c_t[:])
                nc.vector.tensor_tensor(
                    out=diff[:],
                    in0=xb[:, cols],
                    in1=diff[:],
                    op=mybir.AluOpType.subtract,
                )
                sq = pool.tile([P, FEAT_DIM], f32, tag="sq")
                nc.vector.tensor_tensor(
                    out=sq[:], in0=diff[:], in1=diff[:], op=mybir.AluOpType.mult
                )
                s_t = pool.tile([P, 1], f32, tag="s")
                nc.vector.tensor_reduce(
                    out=s_t[:],
                    in_=sq[:],
                    axis=mybir.AxisListType.X,
                    op=mybir.AluOpType.add,
                )
                # torch clamps after masking: clip(d, 1e-12, 1e12) per row
                nc.vector.tensor_scalar(
                    out=s_all[:, t : t + 1],
                    in0=s_t[:],
                    scalar1=1e-12,
                    scalar2=1e12,
                    op0=mybir.AluOpType.max,
                    op1=mybir.AluOpType.min,
                )
            # One DMA for all outputs: dists[n, p] = s_all[p, n]
            nc.sync.dma_start(
                out=out_d[:].rearrange("n p -> p n"),
                in_=s_all[:],
            )
    nc.compile()
    return nc


def kernel(x, labels, centers):
    from concourse.bass_utils import run_bass_kernel_spmd

    x = np.ascontiguousarray(np.asarray(x, dtype=np.float32))
    centers = np.ascontiguousarray(np.asarray(centers, dtype=np.float32))
    labels = np.ascontiguousarray(
        np.asarray(labels).astype(np.int32).reshape(BATCH, 1)
    )

    impl = os.environ.get("CENTERLOSS_IMPL", "v5")
    if ("nc", impl) not in _CACHE:
        builders = {"raw": _build_raw, "v2": _build_v2, "v3": _build_v3, "v4": _build_v4, "v5": _build_v5, "v6": _build_v6, "v7": _build_v7, "v8": _build_v8, "v9": _build_v9, "tile": _build_bass}
        _CACHE[("nc", impl)] = builders[impl]()
    nc = _CACHE[("nc", impl)]

    core_ids = list(range(N_CORES))
    in_maps = [
        {
            "x": x[k * ROWS_PER_CORE : (k + 1) * ROWS_PER_CORE],
            "labels": labels[k * ROWS_PER_CORE : (k + 1) * ROWS_PER_CORE],
            "centers": centers,
        }
        for k in core_ids
    ]

    res = run_bass_kernel_spmd(nc, in_maps, core_ids)
    _CACHE["last_results"] = res

    dists = np.concatenate(
        [
            arr.reshape(-1)
            for k in core_ids
            for name, arr in sorted(res.results[k].items())
            if name.startswith("dists")
        ]
    )
    # Reference clamps after masking: the label entry per row is clipped to
    # [1e-12, 1e12], and the B*(C-1) masked zeros each become 1e-12.
    dists = np.clip(dists, 1e-12, 1e12)
    total = dists.sum(dtype=np.float64) + BATCH * (NUM_CLASSES - 1) * 1e-12
    return np.float32(total / BATCH)

